# revision 63
# baseline (speedup 1.0000x reference)
"""Distributed causal-attention kernel for 8 TRN2 NeuronCores.

Reference computation (B=2, S=2048, E=1024, H=16, D=64):
  qkv = x @ w_qkv; split; rope(q), rope(k); causal softmax attention; out @ w_out

Sharding: data-parallel over batch (cores 0-3 -> batch 0, 4-7 -> batch 1),
tensor-parallel over heads (4 heads per core). Each core computes a partial
[S, E] out-projection; the host sums the 4 partials per batch.

All matmuls run in bf16 on the TensorEngine with fp32 PSUM accumulation.

RoPE uses pair-adjacent head-dim rows (d0,d1,d2,... natural order): the
rotate is then q_rot = q*cos_rep + pairswap(q*sin_signed), computed as two
full-128-partition DVE muls plus one all-SBUF bf16 add; the pair swap runs
on the DMA engines (two partition-stride-2 SBUF->SBUF copies), keeping the
DVE per-chunk cost at ~1.8us vs ~4.1us for the 32-row-block formulation.

Attention runs two independent head streams zipped tile-by-tile
(att_pair2) so the PE stays dense across each stream's exp/mask latency;
the remaining rope pairs are interleaved between attention blocks where
their DVE bursts drain under mask-free score tiles. Input DMAs are split
across the SP/ACT/Pool queues so issue serialization never gates the
rope-swap DMAs.
"""

import sys
import types

sys.path.insert(0, "/opt/trn_rl_repo")

import numpy as np
import ml_dtypes

BF16 = ml_dtypes.bfloat16

B, S, E, H, D = 2, 2048, 1024, 16, 64
N_CORES = 8
HPC = 4           # heads per core
DHC = HPC * D     # 256 head-dims per core
KT = E // 128     # 8 contraction tiles for the qkv projection
ST = S // 128     # 16 sequence tiles of 128
SC = S // 512     # 4 sequence chunks of 512
VW = D + 1        # 65: v plus the ones column (softmax denominator)


def _inject_axon_hooks():
    """Register the NTFF profile hook missing from this image's antenv so
    trace=True (or BASS_TRACE=1 in the env) doesn't crash run_bass_kernel_spmd."""
    if "antenv.axon_hooks" in sys.modules:
        return
    try:
        import antenv  # noqa: F401
    except Exception:
        return
    mod = types.ModuleType("antenv.axon_hooks")
    mod._hook = None

    def set_axon_ntff_profile_hook(h):
        mod._hook = h

    def get_axon_ntff_profile_hook():
        return mod._hook

    mod.set_axon_ntff_profile_hook = set_axon_ntff_profile_hook
    mod.get_axon_ntff_profile_hook = get_axon_ntff_profile_hook
    sys.modules["antenv.axon_hooks"] = mod
    try:
        from trn_agent_boot.trn_boot import _ntff_profile_via_ctypes

        mod._hook = _ntff_profile_via_ctypes("/opt/axon/libaxon_pjrt.so")
    except Exception:
        pass


def build():
    import concourse.bass as bass  # noqa: F401
    import concourse.mybir as mybir
    import concourse.tile as tile
    from concourse import bacc

    f32 = mybir.dt.float32
    bf16 = mybir.dt.bfloat16
    Exp = mybir.ActivationFunctionType.Exp

    nc = bacc.Bacc("TRN2", target_bir_lowering=False, debug=False,
                   num_devices=N_CORES)

    xt_e = nc.dram_tensor("xt", [128, KT * S], bf16, kind="ExternalInput").ap()
    wq_e = nc.dram_tensor("wq", [128, KT * DHC], bf16, kind="ExternalInput").ap()
    wk_e = nc.dram_tensor("wk", [128, KT * DHC], bf16, kind="ExternalInput").ap()
    wv_e = nc.dram_tensor("wv", [128, KT * DHC], bf16, kind="ExternalInput").ap()
    wo_e = nc.dram_tensor("wo", [128, 2 * E], bf16, kind="ExternalInput").ap()
    cr_e = nc.dram_tensor("crep", [128, S], bf16, kind="ExternalInput").ap()
    sr_e = nc.dram_tensor("srep", [128, S], bf16, kind="ExternalInput").ap()
    mk_e = nc.dram_tensor("mask", [128, 128], bf16, kind="ExternalInput").ap()
    out_e = nc.dram_tensor("out", [S, E], bf16, kind="ExternalOutput").ap()

    with tile.TileContext(nc) as tc:
        with (
            tc.tile_pool(name="static", bufs=1) as static,
            tc.tile_pool(name="tap", bufs=3) as tap,
            tc.tile_pool(name="tbp", bufs=3) as tbp,
            tc.tile_pool(name="tsp", bufs=3) as tsp,
            tc.tile_pool(name="expp", bufs=32) as expp,
            tc.tile_pool(name="outp", bufs=4) as outp,
            tc.tile_pool(name="bcp", bufs=2) as bcp,
            tc.tile_pool(name="psacc", bufs=2, space="PSUM") as psacc,
            tc.tile_pool(name="psav", bufs=2, space="PSUM") as psav,
        ):
            xt = static.tile([128, KT * S], bf16, tag="xt")
            wq = static.tile([128, KT * DHC], bf16, tag="wq")
            wk = static.tile([128, KT * DHC], bf16, tag="wk")
            wv = static.tile([128, KT * DHC], bf16, tag="wv")
            wo = static.tile([128, 2 * E], bf16, tag="wo")
            crep = static.tile([128, S], bf16, tag="crep")
            srep = static.tile([128, S], bf16, tag="srep")
            mask = static.tile([128, 128], bf16, tag="mask")
            qs = [static.tile([128, S], bf16, tag=f"q{m}", name=f"q{m}") for m in range(2)]
            ks = [static.tile([128, S], bf16, tag=f"k{m}", name=f"k{m}") for m in range(2)]
            vsb = static.tile([128, ST * HPC, VW], bf16, tag="v")
            attn = [static.tile([128, S], bf16, tag=f"at{m}", name=f"at{m}") for m in range(2)]

            # ---- input DMA: split across SP and ACT queues so issue time
            # (~0.6us per DMA per queue) doesn't serialize the head. First
            # uses first: wv+xt c5=0 (v_tiles 0-4), wq+crep/srep (first rope),
            # then the rest.
            for k2 in range(0, KT, 2):  # wv in 2-k pieces on ACT queue
                nc.scalar.dma_start(
                    out=wv[:, k2 * DHC:(k2 + 2) * DHC],
                    in_=wv_e[:, k2 * DHC:(k2 + 2) * DHC])
            for k in range(KT):  # xt c5=0 per-k on SP queue (gates first work)
                nc.sync.dma_start(out=xt[:, k * S: k * S + 512],
                                  in_=xt_e[:, k * S: k * S + 512])
            nc.sync.dma_start(out=mask[:, :], in_=mk_e[:, :])
            for k2 in range(0, KT, 2):  # wq on ACT queue
                nc.scalar.dma_start(
                    out=wq[:, k2 * DHC:(k2 + 2) * DHC],
                    in_=wq_e[:, k2 * DHC:(k2 + 2) * DHC])
            nc.scalar.dma_start(out=crep[:, :], in_=cr_e[:, :])
            nc.scalar.dma_start(out=srep[:, :], in_=sr_e[:, :])
            # xt c5=1..3 on the Pool queue: issue latency is uncritical there
            # and it keeps SP free for the latency-sensitive rope-swap DMAs
            for k in range(KT):
                nc.gpsimd.dma_start(out=xt[:, k * S + 512: k * S + 1024],
                                    in_=xt_e[:, k * S + 512: k * S + 1024])
            for k4 in range(0, KT, 4):  # wk on ACT queue
                nc.scalar.dma_start(
                    out=wk[:, k4 * DHC:(k4 + 4) * DHC],
                    in_=wk_e[:, k4 * DHC:(k4 + 4) * DHC])
            # xt c5=2+3 are contiguous per k: one 256KB DMA per k
            for k in range(KT):
                nc.gpsimd.dma_start(out=xt[:, k * S + 1024: (k + 1) * S],
                                    in_=xt_e[:, k * S + 1024: (k + 1) * S])
            nc.scalar.dma_start(out=wo[:, :], in_=wo_e[:, :])

            nc.vector.memset(vsb[:, :, D:VW], 1.0)

            # ---- q, k projections + RoPE, [d, s] layout, 512-wide chunks ----
            # m-tile rows: pair-adjacent head dims [hA d0..d63 | hB d0..d63].
            # rot = ps*crep + pairswap(ps*srep_signed); swap via two
            # partition-stride-2 SBUF->SBUF DMAs on the DMA engines.
            def qk_pair_quanta(dst, w, mt, jp):
                # rope a 1024-col chunk pair as two schedulable quanta (one
                # per 512-col half): 8 matmuls + 2 muls each; the second
                # also emits the paired swap DMAs and the combining add.
                state = {}

                def half(hf):
                    if hf == 0:
                        state["ta"] = tap.tile([128, 1024], bf16, tag="ta", name="ta")
                        state["tb"] = tbp.tile([128, 1024], bf16, tag="tb", name="tb")
                        state["tbs"] = tsp.tile([128, 1024], bf16, tag="tbs", name="tbs")
                    ta, tb, tbs = state["ta"], state["tb"], state["tbs"]
                    c5 = 2 * jp + hf
                    ps = psacc.tile([128, 512], f32, tag="qp", name="qp")
                    for k in range(KT):
                        nc.tensor.matmul(
                            ps[:, :],
                            lhsT=w[:, k * DHC + mt * 128: k * DHC + (mt + 1) * 128],
                            rhs=xt[:, k * S + c5 * 512: k * S + (c5 + 1) * 512],
                            start=(k == 0), stop=(k == KT - 1),
                        )
                    hs = slice(hf * 512, (hf + 1) * 512)
                    nc.vector.tensor_mul(ta[:, hs], ps[:, :],
                                         crep[:, c5 * 512:(c5 + 1) * 512])
                    nc.vector.tensor_mul(tb[:, hs], ps[:, :],
                                         srep[:, c5 * 512:(c5 + 1) * 512])
                    if hf == 1:
                        nc.sync.dma_start(out=tbs[0:128:2, :], in_=tb[1:128:2, :])
                        nc.sync.dma_start(out=tbs[1:128:2, :], in_=tb[0:128:2, :])
                        o = dst[mt][:, jp * 1024:(jp + 1) * 1024]
                        nc.vector.tensor_add(o, ta[:, :], tbs[:, :])

                return [lambda: half(0), lambda: half(1)]

            def qk_pair(dst, w, mt, jp):
                for q in qk_pair_quanta(dst, w, mt, jp):
                    q()

            # ---- v = x @ w_v in [s, d] layout --------------------------------
            def v_tiles(st0, st1, pool=None):
                for st in range(st0, st1):
                    ps = (pool or psacc).tile([128, 4, D], f32,
                                              tag="qp" if pool is None else "av",
                                              name="vps")
                    for k in range(KT):
                        nc.tensor.matmul(
                            ps[:, :, :],
                            lhsT=xt[:, k * S + st * 128: k * S + (st + 1) * 128],
                            rhs=wv[:, k * DHC:(k + 1) * DHC],
                            start=(k == 0), stop=(k == KT - 1),
                        )
                    nc.scalar.copy(out=vsb[:, st * HPC:(st + 1) * HPC, 0:D],
                                   in_=ps[:, :, :])

            # ---- causal attention per head, 1024-wide s-chunks ---------------
            # Two passes per (head, chunk): a dense scores+exp streak buffered
            # into SBUF et tiles, then a dense AV streak.
            def av_stream(h, jj, ets):
                # dense AV streak for the 512-wide stream (h, jj), then its
                # normalization chain: attn[d, s] = av[d, s] / av[64, s]
                mt, base = h // 2, (h % 2) * 64
                av = psav.tile([VW, 512], f32, tag="av", name="av")
                n_i = 4 * jj + 4
                for i in range(n_i):
                    et, c0 = ets[i]
                    lo = 512 * (jj % 2)
                    a = max(c0, lo) - lo
                    nc.tensor.matmul(
                        av[:, a:512],
                        lhsT=vsb[:, i * HPC + h, :],
                        rhs=et[:, lo + a:lo + 512],
                        start=(i == 0), stop=(i == n_i - 1),
                    )
                rc = bcp.tile([1, 512], f32, tag="rc")
                bc = bcp.tile([64, 512], f32, tag="bc")
                den = bcp.tile([1, 512], f32, tag="den")
                # custom DVE ops cannot read PSUM (silent garbage on HW):
                # stage the denominator row through SBUF first
                nc.vector.tensor_copy(out=den[:, :], in_=av[D:VW, :])
                nc.vector.reciprocal_approx_fast(rc[:, :], den[:, :])
                nc.gpsimd.partition_broadcast(bc[:, :], rc[:, :])
                nc.vector.tensor_mul(
                    attn[mt][base:base + 64, jj * 512:(jj + 1) * 512],
                    av[0:D, :], bc[:, :])

            def att_passA_range(h, j, i0, i1, ets):
                mt, base = h // 2, (h % 2) * 64
                q_t, k_t = qs[mt], ks[mt]
                for i in range(i0, i1):
                    r = i - 8 * j
                    c0 = 128 * r if r >= 0 else 0
                    sp = psacc.tile([128, 1024], f32, tag="sp", name="sp")
                    for (a, b2) in ((c0, 512), (max(c0, 512), 1024)):
                        if a >= b2:
                            continue
                        nc.tensor.matmul(
                            sp[:, a:b2],
                            lhsT=k_t[base:base + 64, i * 128:(i + 1) * 128],
                            rhs=q_t[base:base + 64, j * 1024 + a: j * 1024 + b2],
                            start=True, stop=True,
                        )
                    et = expp.tile([128, 1024], bf16, tag="e")
                    nc.scalar.activation(
                        et[:, c0:1024], sp[:, c0:1024], Exp, scale=0.125)
                    if r >= 0:
                        nc.vector.tensor_mul(
                            et[:, c0:c0 + 128], et[:, c0:c0 + 128], mask[:, :])
                    ets.append((et, c0))

            def att_pair2(ha, hb, j, side=None):
                # two independent head streams zipped tile-by-tile: when
                # stream A's next matmul would wait on its own exp/mask,
                # stream B's tile keeps the PE queue dense (the HAM clock
                # gate throttles on sub-us PE waits). `side` is a list of
                # independent work quanta (rope halves, v tiles, outproj
                # units) sprinkled between tile pairs so every engine's
                # queue stays mixed rather than bursty.
                side = list(side or [])
                slots = (8 * j + 8) + 2
                per = max(1, -(-len(side) // slots)) if side else 0

                def fill():
                    for _ in range(per):
                        if side:
                            side.pop(0)()

                eA, eB = [], []
                for i in range(8 * j + 4):
                    att_passA_range(ha, j, i, i + 1, eA)
                    att_passA_range(hb, j, i, i + 1, eB)
                    fill()
                av_stream(ha, 2 * j, eA)
                av_stream(hb, 2 * j, eB)
                fill()
                for i in range(8 * j + 4, 8 * j + 8):
                    att_passA_range(ha, j, i, i + 1, eA)
                    att_passA_range(hb, j, i, i + 1, eB)
                    fill()
                av_stream(ha, 2 * j + 1, eA)
                av_stream(hb, 2 * j + 1, eB)
                while side:
                    side.pop(0)()

            # ---- partial out-projection: out = attn.T @ w_out ----------------
            def outproj_unit(st, c2):
                ps = psacc.tile([128, 512], f32, tag="qp", name="ops")
                for kt in range(2):
                    nc.tensor.matmul(
                        ps[:, :],
                        lhsT=attn[kt][:, st * 128:(st + 1) * 128],
                        rhs=wo[:, kt * E + c2 * 512: kt * E + (c2 + 1) * 512],
                        start=(kt == 0), stop=(kt == 1),
                    )
                ot = outp.tile([128, 512], bf16, tag="o")
                if st >= 12 and c2 == 1:
                    # tail block: ACT is idle after the last exp
                    nc.scalar.copy(out=ot[:, :], in_=ps[:, :])
                else:
                    nc.vector.tensor_copy(out=ot[:, :], in_=ps[:, :])
                if st >= 12:
                    # keep the last stores off the Pool queue so its
                    # end-of-kernel drain doesn't extend the teardown
                    eng = nc.sync if c2 == 0 else nc.scalar
                else:
                    eng = nc.sync if (st + c2) % 2 == 0 else nc.gpsimd
                eng.dma_start(
                    out=out_e[st * 128:(st + 1) * 128, c2 * 512:(c2 + 1) * 512],
                    in_=ot[:, :])

            def outproj(st0, st1):
                for st in range(st0, st1):
                    for c2 in range(2):
                        outproj_unit(st, c2)

            # ---- schedule: interleave phases so the PE queue stays dense -----
            # v tiles for s-cols [0:512) need only the first 1MB of xt:
            # they give the PE dense work during the DMA-bound head window
            v_tiles(0, 4, pool=psav)
            qk_pair(qs, wq, 0, 0)
            qk_pair(ks, wk, 0, 0)
            v_tiles(4, 8, pool=psav)
            # remaining ropes and v tiles ride as side quanta inside the
            # attention blocks: chunk-0 attention for heads 0/1 only reads
            # the first roped column-pair
            att_pair2(0, 1, 0)
            # remaining ropes between attention blocks: they fill the PE
            # while each block's trailing exp/normalize chains drain
            qk_pair(qs, wq, 0, 1)
            qk_pair(ks, wk, 0, 1)
            qk_pair(qs, wq, 1, 0)
            qk_pair(ks, wk, 1, 0)
            v_tiles(8, 16)
            att_pair2(0, 1, 1)
            # the last rope pair rides inside the heads-2/3 block as spread
            # quanta: its DVE muls interleave with the block's masks instead
            # of forming a wall in front of them
            side23 = qk_pair_quanta(qs, wq, 1, 1) + qk_pair_quanta(ks, wk, 1, 1)
            att_pair2(2, 3, 0, side=side23)
            e21 = []
            e31 = []
            opq = [(lambda st=st, c2=c2: outproj_unit(st, c2))
                   for st in range(0, 8) for c2 in range(2)]
            for i in range(12):
                att_passA_range(2, 1, i, i + 1, e21)
                att_passA_range(3, 1, i, i + 1, e31)
                if i >= 4:
                    opq.pop(0)()
                    opq.pop(0)()
            av_stream(2, 2, e21)
            av_stream(3, 2, e31)
            opq = [(lambda st=st, c2=c2: outproj_unit(st, c2))
                   for st in range(8, 12) for c2 in range(2)]
            for i in range(12, 16):
                att_passA_range(2, 1, i, i + 1, e21)
                att_passA_range(3, 1, i, i + 1, e31)
                opq.pop(0)()
                opq.pop(0)()
            av_stream(2, 3, e21)
            av_stream(3, 3, e31)
            outproj(12, 16)

    nc.compile()
    return nc


def prep_inputs(x, w_qkv, w_out, freqs_cos, freqs_sin):
    """Shard + pre-tile the full fp32 inputs into 8 per-core in_maps."""
    cosT = np.ascontiguousarray(freqs_cos.T.astype(np.float32))  # [32, S]
    sinT = np.ascontiguousarray(freqs_sin.T.astype(np.float32))
    # pair-adjacent rows: row 2i and 2i+1 both carry freq i; sin is signed
    # (+ on even rows, - on odd) so rot = ps*crep + pairswap(ps*srep).
    cos2 = np.repeat(cosT, 2, axis=0)            # [64, S]
    sin2 = np.repeat(sinT, 2, axis=0).copy()     # [64, S]
    sin2[1::2, :] *= -1.0
    crep = np.tile(cos2, (2, 1)).astype(BF16)    # [128, S]
    srep = np.tile(sin2, (2, 1)).astype(BF16)
    mask = (np.arange(128)[:, None] <= np.arange(128)[None, :]).astype(BF16)

    xt_b = []
    for b in range(B):
        xt = np.ascontiguousarray(x[b].T)  # [E, S]
        xt_b.append(
            xt.reshape(KT, 128, S).transpose(1, 0, 2).reshape(128, KT * S)
            .astype(BF16))

    in_maps = []
    for c in range(N_CORES):
        b, hg = divmod(c, 4)
        cq, ck, cv = [], [], []
        for h in range(HPC):
            gh = hg * HPC + h
            base = gh * D
            perm = np.arange(base, base + D)
            cq.append(perm)
            ck.append(perm + E)
            cv.append(np.arange(base, base + D) + 2 * E)

        def tile_w(cols):
            wc = w_qkv[:, np.concatenate(cols)]  # [E, 256]
            return (wc.reshape(KT, 128, DHC).transpose(1, 0, 2)
                    .reshape(128, KT * DHC).astype(BF16))

        wo_c = w_out[hg * DHC:(hg + 1) * DHC, :]  # [256, E]
        wo_p = (wo_c.reshape(2, 128, E).transpose(1, 0, 2)
                .reshape(128, 2 * E).astype(BF16))
        in_maps.append({
            "xt": xt_b[b],
            "wq": tile_w(cq),
            "wk": tile_w(ck),
            "wv": tile_w(cv),
            "wo": wo_p,
            "crep": crep,
            "srep": srep,
            "mask": mask,
        })
    return in_maps


_CACHE = {}


def _get_nc():
    if "nc" not in _CACHE:
        _inject_axon_hooks()
        _CACHE["nc"] = build()
    return _CACHE["nc"]


def kernel(x, w_qkv, w_out, freqs_cos, freqs_sin):
    from concourse.bass_utils import run_bass_kernel_spmd

    nc = _get_nc()
    in_maps = prep_inputs(
        np.asarray(x, dtype=np.float32),
        np.asarray(w_qkv, dtype=np.float32),
        np.asarray(w_out, dtype=np.float32),
        np.asarray(freqs_cos, dtype=np.float32),
        np.asarray(freqs_sin, dtype=np.float32),
    )
    res = run_bass_kernel_spmd(nc, in_maps, core_ids=list(range(N_CORES)))
    parts = [np.asarray(res.results[c]["out"], dtype=np.float32)
             for c in range(N_CORES)]
    out = np.stack([
        parts[0] + parts[1] + parts[2] + parts[3],
        parts[4] + parts[5] + parts[6] + parts[7],
    ]).astype(np.float32)
    return out


# revision 83
# speedup vs baseline: 1.0109x; 1.0109x over previous
"""Distributed causal-attention kernel for 8 TRN2 NeuronCores.

Reference computation (B=2, S=2048, E=1024, H=16, D=64):
  qkv = x @ w_qkv; split; rope(q), rope(k); causal softmax attention; out @ w_out

Sharding: data-parallel over batch (cores 0-3 -> batch 0, 4-7 -> batch 1),
tensor-parallel over heads (4 heads per core). Each core computes a partial
[S, E] out-projection; the host sums the 4 partials per batch.

All matmuls run in bf16 on the TensorEngine with fp32 PSUM accumulation.

RoPE uses pair-adjacent head-dim rows (d0,d1,d2,... natural order): the
rotate is then q_rot = q*cos_rep + pairswap(q*sin_signed), computed as two
full-128-partition DVE muls plus one all-SBUF bf16 add; the pair swap runs
on the DMA engines (two partition-stride-2 SBUF->SBUF copies), keeping the
DVE per-chunk cost at ~1.8us vs ~4.1us for the 32-row-block formulation.

Attention runs two independent head streams zipped tile-by-tile
(att_pair2) so the PE stays dense across each stream's exp/mask latency;
the remaining rope pairs are interleaved between attention blocks where
their DVE bursts drain under mask-free score tiles. Input DMAs are split
across the SP/ACT/Pool queues so issue serialization never gates the
rope-swap DMAs.
"""

import sys
import types

sys.path.insert(0, "/opt/trn_rl_repo")

import numpy as np
import ml_dtypes

BF16 = ml_dtypes.bfloat16

B, S, E, H, D = 2, 2048, 1024, 16, 64
N_CORES = 8
HPC = 4           # heads per core
DHC = HPC * D     # 256 head-dims per core
KT = E // 128     # 8 contraction tiles for the qkv projection
ST = S // 128     # 16 sequence tiles of 128
SC = S // 512     # 4 sequence chunks of 512
VW = D + 1        # 65: v plus the ones column (softmax denominator)


def _inject_axon_hooks():
    """Register the NTFF profile hook missing from this image's antenv so
    trace=True (or BASS_TRACE=1 in the env) doesn't crash run_bass_kernel_spmd."""
    if "antenv.axon_hooks" in sys.modules:
        return
    try:
        import antenv  # noqa: F401
    except Exception:
        return
    mod = types.ModuleType("antenv.axon_hooks")
    mod._hook = None

    def set_axon_ntff_profile_hook(h):
        mod._hook = h

    def get_axon_ntff_profile_hook():
        return mod._hook

    mod.set_axon_ntff_profile_hook = set_axon_ntff_profile_hook
    mod.get_axon_ntff_profile_hook = get_axon_ntff_profile_hook
    sys.modules["antenv.axon_hooks"] = mod
    try:
        from trn_agent_boot.trn_boot import _ntff_profile_via_ctypes

        mod._hook = _ntff_profile_via_ctypes("/opt/axon/libaxon_pjrt.so")
    except Exception:
        pass


def build():
    import concourse.bass as bass  # noqa: F401
    import concourse.mybir as mybir
    import concourse.tile as tile
    from concourse import bacc

    f32 = mybir.dt.float32
    bf16 = mybir.dt.bfloat16
    Exp = mybir.ActivationFunctionType.Exp

    nc = bacc.Bacc("TRN2", target_bir_lowering=False, debug=False,
                   num_devices=N_CORES)

    xt_e = nc.dram_tensor("xt", [128, KT * S], bf16, kind="ExternalInput").ap()
    wq_e = nc.dram_tensor("wq", [128, KT * DHC], bf16, kind="ExternalInput").ap()
    wk_e = nc.dram_tensor("wk", [128, KT * DHC], bf16, kind="ExternalInput").ap()
    wv_e = nc.dram_tensor("wv", [128, KT * DHC], bf16, kind="ExternalInput").ap()
    wo_e = nc.dram_tensor("wo", [128, 2 * E], bf16, kind="ExternalInput").ap()
    cr_e = nc.dram_tensor("crep", [128, S], bf16, kind="ExternalInput").ap()
    sr_e = nc.dram_tensor("srep", [128, S], bf16, kind="ExternalInput").ap()
    mk_e = nc.dram_tensor("mask", [128, 128], bf16, kind="ExternalInput").ap()
    out_e = nc.dram_tensor("out", [S, E], bf16, kind="ExternalOutput").ap()

    with tile.TileContext(nc) as tc:
        with (
            tc.tile_pool(name="static", bufs=1) as static,
            tc.tile_pool(name="tap", bufs=3) as tap,
            tc.tile_pool(name="tbp", bufs=3) as tbp,
            tc.tile_pool(name="tsp", bufs=3) as tsp,
            tc.tile_pool(name="expp", bufs=32) as expp,
            tc.tile_pool(name="outp", bufs=4) as outp,
            tc.tile_pool(name="bcp", bufs=2) as bcp,
            tc.tile_pool(name="psacc", bufs=2, space="PSUM") as psacc,
            tc.tile_pool(name="psav", bufs=2, space="PSUM") as psav,
        ):
            xt = static.tile([128, KT * S], bf16, tag="xt")
            wq = static.tile([128, KT * DHC], bf16, tag="wq")
            wk = static.tile([128, KT * DHC], bf16, tag="wk")
            wv = static.tile([128, KT * DHC], bf16, tag="wv")
            wo = static.tile([128, 2 * E], bf16, tag="wo")
            crep = static.tile([128, S], bf16, tag="crep")
            srep = static.tile([128, S], bf16, tag="srep")
            mask = static.tile([128, 128], bf16, tag="mask")
            qs = [static.tile([128, S], bf16, tag=f"q{m}", name=f"q{m}") for m in range(2)]
            ks = [static.tile([128, S], bf16, tag=f"k{m}", name=f"k{m}") for m in range(2)]
            vsb = static.tile([128, ST * HPC, VW], bf16, tag="v")
            attn = [static.tile([128, S], bf16, tag=f"at{m}", name=f"at{m}") for m in range(2)]

            # ---- input DMA: split across SP and ACT queues so issue time
            # (~0.6us per DMA per queue) doesn't serialize the head. First
            # uses first: wv+xt c5=0 (v_tiles 0-4), wq+crep/srep (first rope),
            # then the rest.
            for k2 in range(0, KT, 2):  # wv in 2-k pieces on ACT queue
                nc.scalar.dma_start(
                    out=wv[:, k2 * DHC:(k2 + 2) * DHC],
                    in_=wv_e[:, k2 * DHC:(k2 + 2) * DHC])
            for k in range(KT):  # xt c5=0 per-k on SP queue (gates first work)
                nc.sync.dma_start(out=xt[:, k * S: k * S + 512],
                                  in_=xt_e[:, k * S: k * S + 512])
            nc.sync.dma_start(out=mask[:, :], in_=mk_e[:, :])
            for k2 in range(0, KT, 2):  # wq on ACT queue
                nc.scalar.dma_start(
                    out=wq[:, k2 * DHC:(k2 + 2) * DHC],
                    in_=wq_e[:, k2 * DHC:(k2 + 2) * DHC])
            nc.scalar.dma_start(out=crep[:, :], in_=cr_e[:, :])
            nc.scalar.dma_start(out=srep[:, :], in_=sr_e[:, :])
            # xt c5=1..3 on the Pool queue: issue latency is uncritical there
            # and it keeps SP free for the latency-sensitive rope-swap DMAs
            for k in range(KT):
                nc.gpsimd.dma_start(out=xt[:, k * S + 512: k * S + 1024],
                                    in_=xt_e[:, k * S + 512: k * S + 1024])
            for k4 in range(0, KT, 4):  # wk on ACT queue
                nc.scalar.dma_start(
                    out=wk[:, k4 * DHC:(k4 + 4) * DHC],
                    in_=wk_e[:, k4 * DHC:(k4 + 4) * DHC])
            # xt c5=2+3 are contiguous per k: one 256KB DMA per k
            for k in range(KT):
                nc.gpsimd.dma_start(out=xt[:, k * S + 1024: (k + 1) * S],
                                    in_=xt_e[:, k * S + 1024: (k + 1) * S])
            nc.scalar.dma_start(out=wo[:, :], in_=wo_e[:, :])

            nc.vector.memset(vsb[:, :, D:VW], 1.0)

            # ---- q, k projections + RoPE, [d, s] layout, 512-wide chunks ----
            # m-tile rows: pair-adjacent head dims [hA d0..d63 | hB d0..d63].
            # rot = ps*crep + pairswap(ps*srep_signed); swap via two
            # partition-stride-2 SBUF->SBUF DMAs on the DMA engines.
            def qk_pair_quanta(dst, w, mt, jp, ptag="qp"):
                # rope a 1024-col chunk pair as two schedulable quanta (one
                # per 512-col half): 8 matmuls + 2 muls each; the second
                # also emits the paired swap DMAs and the combining add.
                # (ptag="av" - borrowing the AV PSUM ring - measured slower:
                # it adds a WAW on the previous block's normalize chain.)
                state = {}

                def half(hf):
                    if hf == 0:
                        state["ta"] = tap.tile([128, 1024], bf16, tag="ta", name="ta")
                        state["tb"] = tbp.tile([128, 1024], bf16, tag="tb", name="tb")
                        state["tbs"] = tsp.tile([128, 1024], bf16, tag="tbs", name="tbs")
                    ta, tb, tbs = state["ta"], state["tb"], state["tbs"]
                    c5 = 2 * jp + hf
                    pool = psav if ptag == "av" else psacc
                    ps = pool.tile([128, 512], f32, tag=ptag, name="qp")
                    for k in range(KT):
                        nc.tensor.matmul(
                            ps[:, :],
                            lhsT=w[:, k * DHC + mt * 128: k * DHC + (mt + 1) * 128],
                            rhs=xt[:, k * S + c5 * 512: k * S + (c5 + 1) * 512],
                            start=(k == 0), stop=(k == KT - 1),
                        )
                    hs = slice(hf * 512, (hf + 1) * 512)
                    nc.vector.tensor_mul(ta[:, hs], ps[:, :],
                                         crep[:, c5 * 512:(c5 + 1) * 512])
                    nc.vector.tensor_mul(tb[:, hs], ps[:, :],
                                         srep[:, c5 * 512:(c5 + 1) * 512])
                    if hf == 1:
                        nc.sync.dma_start(out=tbs[0:128:2, :], in_=tb[1:128:2, :])
                        nc.sync.dma_start(out=tbs[1:128:2, :], in_=tb[0:128:2, :])
                        o = dst[mt][:, jp * 1024:(jp + 1) * 1024]
                        nc.vector.tensor_add(o, ta[:, :], tbs[:, :])

                return [lambda: half(0), lambda: half(1)]

            def qk_pair(dst, w, mt, jp, ptag="qp"):
                for q in qk_pair_quanta(dst, w, mt, jp, ptag):
                    q()

            # ---- v = x @ w_v in [s, d] layout --------------------------------
            def v_tiles(st0, st1, pool=None):
                for st in range(st0, st1):
                    ps = (pool or psacc).tile([128, 4, D], f32,
                                              tag="qp" if pool is None else "av",
                                              name="vps")
                    for k in range(KT):
                        nc.tensor.matmul(
                            ps[:, :, :],
                            lhsT=xt[:, k * S + st * 128: k * S + (st + 1) * 128],
                            rhs=wv[:, k * DHC:(k + 1) * DHC],
                            start=(k == 0), stop=(k == KT - 1),
                        )
                    nc.scalar.copy(out=vsb[:, st * HPC:(st + 1) * HPC, 0:D],
                                   in_=ps[:, :, :])

            # ---- causal attention per head, 1024-wide s-chunks ---------------
            # Two passes per (head, chunk): a dense scores+exp streak buffered
            # into SBUF et tiles, then a dense AV streak.
            def av_stream(h, jj, ets):
                # dense AV streak for the 512-wide stream (h, jj), then its
                # normalization chain: attn[d, s] = av[d, s] / av[64, s]
                mt, base = h // 2, (h % 2) * 64
                av = psav.tile([VW, 512], f32, tag="av", name="av")
                n_i = 4 * jj + 4
                for i in range(n_i):
                    et, c0 = ets[i]
                    lo = 512 * (jj % 2)
                    a = max(c0, lo) - lo
                    nc.tensor.matmul(
                        av[:, a:512],
                        lhsT=vsb[:, i * HPC + h, :],
                        rhs=et[:, lo + a:lo + 512],
                        start=(i == 0), stop=(i == n_i - 1),
                    )
                rc = bcp.tile([1, 512], f32, tag="rc")
                bc = bcp.tile([64, 512], f32, tag="bc")
                den = bcp.tile([1, 512], f32, tag="den")
                # custom DVE ops cannot read PSUM (silent garbage on HW):
                # stage the denominator row through SBUF first
                nc.vector.tensor_copy(out=den[:, :], in_=av[D:VW, :])
                nc.vector.reciprocal_approx_fast(rc[:, :], den[:, :])
                nc.gpsimd.partition_broadcast(bc[:, :], rc[:, :])
                nc.vector.tensor_mul(
                    attn[mt][base:base + 64, jj * 512:(jj + 1) * 512],
                    av[0:D, :], bc[:, :])

            def att_passA_range(h, j, i0, i1, ets):
                mt, base = h // 2, (h % 2) * 64
                q_t, k_t = qs[mt], ks[mt]
                for i in range(i0, i1):
                    r = i - 8 * j
                    c0 = 128 * r if r >= 0 else 0
                    sp = psacc.tile([128, 1024], f32, tag="sp", name="sp")
                    for (a, b2) in ((c0, 512), (max(c0, 512), 1024)):
                        if a >= b2:
                            continue
                        nc.tensor.matmul(
                            sp[:, a:b2],
                            lhsT=k_t[base:base + 64, i * 128:(i + 1) * 128],
                            rhs=q_t[base:base + 64, j * 1024 + a: j * 1024 + b2],
                            start=True, stop=True,
                        )
                    et = expp.tile([128, 1024], bf16, tag="e")
                    nc.scalar.activation(
                        et[:, c0:1024], sp[:, c0:1024], Exp, scale=0.125)
                    if r >= 0:
                        nc.vector.tensor_mul(
                            et[:, c0:c0 + 128], et[:, c0:c0 + 128], mask[:, :])
                    ets.append((et, c0))

            def att_pair2(ha, hb, j, side=None):
                # two independent head streams zipped tile-by-tile: when
                # stream A's next matmul would wait on its own exp/mask,
                # stream B's tile keeps the PE queue dense (the HAM clock
                # gate throttles on sub-us PE waits). `side` is a list of
                # independent work quanta (rope halves, v tiles, outproj
                # units) sprinkled between tile pairs so every engine's
                # queue stays mixed rather than bursty.
                side = list(side or [])
                slots = (8 * j + 8) + 2
                per = max(1, -(-len(side) // slots)) if side else 0

                def fill():
                    for _ in range(per):
                        if side:
                            side.pop(0)()

                eA, eB = [], []
                for i in range(8 * j + 4):
                    att_passA_range(ha, j, i, i + 1, eA)
                    att_passA_range(hb, j, i, i + 1, eB)
                    fill()
                av_stream(ha, 2 * j, eA)
                av_stream(hb, 2 * j, eB)
                fill()
                for i in range(8 * j + 4, 8 * j + 8):
                    att_passA_range(ha, j, i, i + 1, eA)
                    att_passA_range(hb, j, i, i + 1, eB)
                    fill()
                av_stream(ha, 2 * j + 1, eA)
                av_stream(hb, 2 * j + 1, eB)
                while side:
                    side.pop(0)()

            # ---- partial out-projection: out = attn.T @ w_out ----------------
            def outproj_unit(st, c2):
                ps = psacc.tile([128, 512], f32, tag="qp", name="ops")
                for kt in range(2):
                    nc.tensor.matmul(
                        ps[:, :],
                        lhsT=attn[kt][:, st * 128:(st + 1) * 128],
                        rhs=wo[:, kt * E + c2 * 512: kt * E + (c2 + 1) * 512],
                        start=(kt == 0), stop=(kt == 1),
                    )
                ot = outp.tile([128, 512], bf16, tag="o")
                if st >= 12 and c2 == 1:
                    # tail block: ACT is idle after the last exp
                    nc.scalar.copy(out=ot[:, :], in_=ps[:, :])
                else:
                    nc.vector.tensor_copy(out=ot[:, :], in_=ps[:, :])
                if st >= 12:
                    # keep the last stores off the Pool queue so its
                    # end-of-kernel drain doesn't extend the teardown
                    eng = nc.sync if c2 == 0 else nc.scalar
                else:
                    eng = nc.sync if (st + c2) % 2 == 0 else nc.gpsimd
                eng.dma_start(
                    out=out_e[st * 128:(st + 1) * 128, c2 * 512:(c2 + 1) * 512],
                    in_=ot[:, :])

            def outproj(st0, st1):
                for st in range(st0, st1):
                    for c2 in range(2):
                        outproj_unit(st, c2)

            # ---- schedule: interleave phases so the PE queue stays dense -----
            # v tiles for s-cols [0:512) need only the first 1MB of xt:
            # they give the PE dense work during the DMA-bound head window
            v_tiles(0, 4, pool=psav)
            qk_pair(qs, wq, 0, 0)
            qk_pair(ks, wk, 0, 0)
            v_tiles(4, 8, pool=psav)
            # remaining ropes and v tiles ride as side quanta inside the
            # attention blocks: chunk-0 attention for heads 0/1 only reads
            # the first roped column-pair
            att_pair2(0, 1, 0)
            # remaining ropes between attention blocks: they fill the PE
            # while each block's trailing exp/normalize chains drain
            qk_pair(qs, wq, 0, 1)
            qk_pair(ks, wk, 0, 1)
            qk_pair(qs, wq, 1, 0)
            qk_pair(ks, wk, 1, 0)
            v_tiles(8, 16)
            att_pair2(0, 1, 1)
            # the last rope pair rides inside the heads-2/3 block as spread
            # quanta: its DVE muls interleave with the block's masks instead
            # of forming a wall in front of them
            side23 = qk_pair_quanta(qs, wq, 1, 1) + qk_pair_quanta(ks, wk, 1, 1)
            att_pair2(2, 3, 0, side=side23)
            e21 = []
            e31 = []
            opq = [(lambda st=st, c2=c2: outproj_unit(st, c2))
                   for st in range(0, 8) for c2 in range(2)]
            for i in range(12):
                att_passA_range(2, 1, i, i + 1, e21)
                att_passA_range(3, 1, i, i + 1, e31)
                if i >= 4:
                    opq.pop(0)()
                    opq.pop(0)()
            av_stream(2, 2, e21)
            av_stream(3, 2, e31)
            opq = [(lambda st=st, c2=c2: outproj_unit(st, c2))
                   for st in range(8, 12) for c2 in range(2)]
            for i in range(12, 16):
                att_passA_range(2, 1, i, i + 1, e21)
                att_passA_range(3, 1, i, i + 1, e31)
                opq.pop(0)()
                opq.pop(0)()
            av_stream(2, 3, e21)
            av_stream(3, 3, e31)
            outproj(12, 16)

    nc.compile()
    return nc


def prep_inputs(x, w_qkv, w_out, freqs_cos, freqs_sin):
    """Shard + pre-tile the full fp32 inputs into 8 per-core in_maps."""
    cosT = np.ascontiguousarray(freqs_cos.T.astype(np.float32))  # [32, S]
    sinT = np.ascontiguousarray(freqs_sin.T.astype(np.float32))
    # pair-adjacent rows: row 2i and 2i+1 both carry freq i; sin is signed
    # (+ on even rows, - on odd) so rot = ps*crep + pairswap(ps*srep).
    cos2 = np.repeat(cosT, 2, axis=0)            # [64, S]
    sin2 = np.repeat(sinT, 2, axis=0).copy()     # [64, S]
    sin2[1::2, :] *= -1.0
    crep = np.tile(cos2, (2, 1)).astype(BF16)    # [128, S]
    srep = np.tile(sin2, (2, 1)).astype(BF16)
    mask = (np.arange(128)[:, None] <= np.arange(128)[None, :]).astype(BF16)

    xt_b = []
    for b in range(B):
        xt = np.ascontiguousarray(x[b].T)  # [E, S]
        xt_b.append(
            xt.reshape(KT, 128, S).transpose(1, 0, 2).reshape(128, KT * S)
            .astype(BF16))

    in_maps = []
    for c in range(N_CORES):
        b, hg = divmod(c, 4)
        cq, ck, cv = [], [], []
        for h in range(HPC):
            gh = hg * HPC + h
            base = gh * D
            perm = np.arange(base, base + D)
            cq.append(perm)
            ck.append(perm + E)
            cv.append(np.arange(base, base + D) + 2 * E)

        def tile_w(cols):
            wc = w_qkv[:, np.concatenate(cols)]  # [E, 256]
            return (wc.reshape(KT, 128, DHC).transpose(1, 0, 2)
                    .reshape(128, KT * DHC).astype(BF16))

        wo_c = w_out[hg * DHC:(hg + 1) * DHC, :]  # [256, E]
        wo_p = (wo_c.reshape(2, 128, E).transpose(1, 0, 2)
                .reshape(128, 2 * E).astype(BF16))
        in_maps.append({
            "xt": xt_b[b],
            "wq": tile_w(cq),
            "wk": tile_w(ck),
            "wv": tile_w(cv),
            "wo": wo_p,
            "crep": crep,
            "srep": srep,
            "mask": mask,
        })
    return in_maps


_CACHE = {}


def _get_nc():
    if "nc" not in _CACHE:
        _inject_axon_hooks()
        _CACHE["nc"] = build()
    return _CACHE["nc"]


def kernel(x, w_qkv, w_out, freqs_cos, freqs_sin):
    from concourse.bass_utils import run_bass_kernel_spmd

    nc = _get_nc()
    in_maps = prep_inputs(
        np.asarray(x, dtype=np.float32),
        np.asarray(w_qkv, dtype=np.float32),
        np.asarray(w_out, dtype=np.float32),
        np.asarray(freqs_cos, dtype=np.float32),
        np.asarray(freqs_sin, dtype=np.float32),
    )
    res = run_bass_kernel_spmd(nc, in_maps, core_ids=list(range(N_CORES)))
    parts = [np.asarray(res.results[c]["out"], dtype=np.float32)
             for c in range(N_CORES)]
    out = np.stack([
        parts[0] + parts[1] + parts[2] + parts[3],
        parts[4] + parts[5] + parts[6] + parts[7],
    ]).astype(np.float32)
    return out
